# revision 13
# baseline (speedup 1.0000x reference)
"""GAT-with-LSTM-gates kernel for Trainium2, SPMD over 8 NeuronCores.

Problem: B=16 graphs, N=1024 nodes, D=128 features.
    h   = x @ Ww.T + Wb
    e   = (h @ A) @ h.T;  e_sym = e + e.T  (== h @ (A + A.T) @ h.T)
    s   = where(adj > 0, e_sym, 0)
    att = softmax(s, axis=1) * adj
    h'  = relu(att @ h)
    ic/fc/oc = sigmoid(h' @ w*_u + x @ w*_x)        (scalar per node)
    out = oc * tanh(ic * h' + fc * x)

Sharding: data-parallel over B; 2 graphs per core; params replicated.

Device-side formulation (per graph), all in "transposed" layouts so the
softmax axis is the free dimension:
    hT[d, n]    = Ww @ x.T + Wb                (fp32r matmul)
    hAsT[l, n]  = (A + A.T) @ hT               (fp32r matmul)
    e[c, a]     = e_sym[c, a]  (symmetric)     (fp32r matmul)
    s[c, a]     = e * adjT      (adjT = adj.T in bf16 — exact 0/1 mask)
    p[c, a]     = exp(s);  Z[c] = rowsum(p) fused into the ACT pass
                   (masked entries contribute exp(0)=1, matching the
                   reference softmax denominator; no max-shift: |e| < 30)
    q[c, a]     = p * adjT      (re-mask; the attention numerator)
    h'T[d, a]   = sum_c (h[c, d]/Z[c]) * q[c, a]   (1/Z folded into the
                   small h matrix, not the [N,N] attention matrix)
    h'T         = relu(h'T)
    GT[3, n]    = U.T @ h'T + Xw.T @ xT;  gates = sigmoid(GT)
                   (sigmoid via 0.5*tanh(0.5 z)+0.5 to stay in the exp/tanh
                   ACT table set)
    out[a, d]   = oc * tanh(ic * h'_nat + fc * x_nat)

float32r (TF32-class PE fast path, 1 cycle/row vs 4 for fp32) is used for
all the N=512 matmuls; operands are rounded at their producer ops.
"""

import contextlib

import numpy as np

import concourse.bacc as bacc
import concourse.bass as bass
import concourse.mybir as mybir
import concourse.tile as tile
from concourse.bass_utils import run_bass_kernel_spmd

F32 = mybir.dt.float32
F32R = mybir.dt.float32r
BF16 = mybir.dt.bfloat16
AF = mybir.ActivationFunctionType
OP = mybir.AluOpType

B, N, D = 16, 1024, 128
NCORES = 8
GPC = B // NCORES  # graphs per core
NT = N // 128      # 8 row-strips of the [N, N] score matrix

# Of the 8 re-mask (q = p * adjT) tiles per graph, how many run on GPSIMD
# instead of DVE (load balance).
Q_TILES_ON_GPSIMD = 4


def _build_program(reps=1):
    """reps>1 wraps the whole per-call body in a hardware loop — used only
    for benchmarking (amortizes the host->device dispatch overhead)."""
    nc = bacc.Bacc(None, enable_partition_id=False)

    xT = nc.dram_tensor("xT", [GPC, D, N], F32, kind="ExternalInput")
    xn = nc.dram_tensor("xn", [GPC, N, D], F32, kind="ExternalInput")
    adjT = nc.dram_tensor("adjT", [GPC, N, N], BF16, kind="ExternalInput")
    # all replicated params in one tensor -> one DMA -> one sync wait
    # columns: [WwT(128) | As(128) | I128(128) | Wb(1) | U(3) | Xw(3)]
    consts_d = nc.dram_tensor("consts", [D, 391], F32, kind="ExternalInput")
    out = nc.dram_tensor("out", [GPC, N, D], F32, kind="ExternalOutput")

    with tile.TileContext(nc) as tc:
        with (
            tc.tile_pool(name="const", bufs=1) as constp,
            tc.tile_pool(name="big", bufs=2) as big,
            tc.tile_pool(name="adjp", bufs=3) as adjp,
            tc.tile_pool(name="qp", bufs=3) as qp,
            tc.tile_pool(name="small", bufs=2) as small,
            tc.tile_pool(name="ps_big", bufs=2, space="PSUM") as ps_big,
            tc.tile_pool(name="ps_hp", bufs=1, space="PSUM") as ps_hp,
            tc.tile_pool(name="ps_small", bufs=2, space="PSUM") as ps_small,
        ):
            # ---- constants (loaded once, single DMA) ----
            consts = constp.tile([D, 391], F32, name="consts_sb")
            nc.sync.dma_start(out=consts[:], in_=consts_d[:])
            Wb = consts[:, 384:385]
            I128 = consts[:, 256:384]
            # rounded copies for the fp32r matmuls
            cr = constp.tile([D, 384 + 6], F32R, name="consts_r")
            nc.vector.tensor_copy(cr[:, 0:384], consts[:, 0:384])
            nc.vector.tensor_copy(cr[:, 384:390], consts[:, 385:391])
            WwT_r = cr[:, 0:128]
            As_r = cr[:, 128:256]
            I128_r = cr[:, 256:384]
            U_r = cr[:, 384:387]
            Xw_r = cr[:, 387:390]

            loop_ctx = tc.For_i(0, reps, 1) if reps > 1 else contextlib.nullcontext()
            with loop_ctx:
              for g in range(GPC):
                # ---- load x in both layouts; round xT for fp32r use ----
                xT_sb = big.tile([D, N], F32, name="xT_sb", tag="xT")
                nc.sync.dma_start(out=xT_sb[:], in_=xT[g])
                xT_r = big.tile([D, N], F32R, name="xT_r", tag="xTr")
                nc.gpsimd.tensor_copy(xT_r[:], xT_sb[:])

                # ---- hT = Ww @ x.T + Wb  (fp32r) ----
                hT_ps = ps_big.tile([D, 2, 512], F32, name="hT_ps", tag="bigps")
                for k in range(2):
                    nc.tensor.matmul(
                        hT_ps[:, k, :], WwT_r[:], xT_r[:, k * 512:(k + 1) * 512],
                        start=True, stop=True,
                    )
                hT = big.tile([D, N], F32R, name="hT", tag="hT")
                for k in range(2):
                    nc.scalar.activation(
                        hT[:, k * 512:(k + 1) * 512], hT_ps[:, k, :],
                        AF.Identity, bias=Wb,
                    )

                # ---- hAsT = (A + A.T) @ hT  (fp32r) ----
                hAsT_ps = ps_big.tile([D, 2, 512], F32, name="hAsT_ps", tag="bigps")
                for k in range(2):
                    nc.tensor.matmul(
                        hAsT_ps[:, k, :], As_r[:], hT[:, k * 512:(k + 1) * 512],
                        start=True, stop=True,
                    )
                hAsT = big.tile([D, N], F32R, name="hAsT", tag="hAsT")
                for k in range(2):
                    nc.vector.tensor_copy(
                        hAsT[:, k * 512:(k + 1) * 512], hAsT_ps[:, k, :]
                    )

                # ---- h in natural layout (for the aggregation lhsT) ----
                h_nd = big.tile([128, NT, D], F32, name="h_nd", tag="h_nd")
                h_s = big.tile([128, NT, D], F32R, name="h_s", tag="h_s")
                for ci in range(NT):
                    tr_ps = ps_small.tile([128, D], F32R, name="tr_ps", tag="smallps")
                    nc.tensor.transpose(
                        tr_ps[:], hT[:, ci * 128:(ci + 1) * 128], I128_r
                    )
                    nc.vector.tensor_copy(h_nd[:, ci, :], tr_ps[:])

                # ---- attention, one 128-row strip of scores at a time ----
                hp_ps = ps_hp.tile([D, 2, 512], F32, name="hp_ps", tag="hpps")
                for ci in range(NT):
                    adj_sb = adjp.tile([128, N], BF16, name="adj_sb", tag="adj")
                    nc.sync.dma_start(
                        out=adj_sb[:], in_=adjT[g, ci * 128:(ci + 1) * 128, :]
                    )
                    e_ps = ps_big.tile([128, 2, 512], F32, name="e_ps", tag="bigps")
                    for k in range(2):
                        nc.tensor.matmul(
                            e_ps[:, k, :],
                            hAsT[:, ci * 128:(ci + 1) * 128],
                            hT[:, k * 512:(k + 1) * 512],
                            start=True, stop=True,
                        )
                    # mask -> exp (Z fused) -> re-mask
                    s_sb = qp.tile([128, N], F32, name="s_sb", tag="s")
                    nc.vector.tensor_tensor(
                        s_sb[:], e_ps.rearrange("p a b -> p (a b)"), adj_sb[:],
                        OP.mult,
                    )
                    p_sb = qp.tile([128, N], F32, name="p_sb", tag="p")
                    Z = small.tile([128, 1], F32, name="Z", tag="Z")
                    nc.scalar.activation(p_sb[:], s_sb[:], AF.Exp, accum_out=Z[:])

                    q_sb = qp.tile([128, N], F32R, name="q_sb", tag="q")
                    if ci < Q_TILES_ON_GPSIMD:
                        nc.gpsimd.tensor_tensor(q_sb[:], p_sb[:], adj_sb[:], OP.mult)
                    else:
                        nc.vector.scalar_tensor_tensor(
                            out=q_sb[:], in0=p_sb[:], scalar=1.0, in1=adj_sb[:],
                            op0=OP.mult, op1=OP.mult,
                        )
                    R = small.tile([128, 1], F32, name="R", tag="R")
                    nc.vector.reciprocal(R[:], Z[:])
                    nc.vector.tensor_scalar(
                        h_s[:, ci, :], h_nd[:, ci, :], R[:], None, OP.mult
                    )
                    # accumulate h'T += h_s[ci].T @ q[ci]
                    for k in range(2):
                        nc.tensor.matmul(
                            hp_ps[:, k, :],
                            h_s[:, ci, :],
                            q_sb[:, k * 512:(k + 1) * 512],
                            start=(ci == 0), stop=(ci == NT - 1),
                        )

                # ---- h' = relu ----
                hp = big.tile([D, N], F32R, name="hp", tag="hp")
                for k in range(2):
                    nc.scalar.activation(
                        hp[:, k * 512:(k + 1) * 512], hp_ps[:, k, :], AF.Relu
                    )

                # ---- gates: GT = U.T @ h'T + Xw.T @ xT; sigmoid via tanh ----
                gt = small.tile([32, N], F32, name="gt", tag="gt")
                for k in range(2):
                    gt_ps = ps_small.tile([128, 512], F32, name="gt_ps", tag="smallps")
                    nc.tensor.matmul(
                        gt_ps[0:3, :], U_r[:], hp[:, k * 512:(k + 1) * 512],
                        start=True, stop=False,
                    )
                    nc.tensor.matmul(
                        gt_ps[0:3, :], Xw_r[:], xT_r[:, k * 512:(k + 1) * 512],
                        start=False, stop=True,
                    )
                    nc.scalar.activation(
                        gt[0:3, k * 512:(k + 1) * 512], gt_ps[0:3, :],
                        AF.Tanh, scale=0.5,
                    )
                nc.vector.tensor_scalar(
                    gt[0:3, :], gt[0:3, :], 0.5, 0.5, OP.mult, OP.add
                )

                # ---- final elementwise stage, in natural [node, feat] layout --
                xn_sb = big.tile([128, NT, D], F32, name="xn_sb", tag="xn")
                nc.sync.dma_start(
                    out=xn_sb[:], in_=xn[g].rearrange("(t p) d -> p t d", p=128)
                )
                w_all = big.tile([128, N], F32, name="w_all", tag="w_all")
                t_all = big.tile([128, N], F32, name="t_all", tag="t_all")
                out_sb = big.tile([128, NT, D], F32, name="out_sb", tag="out_sb")
                gn = small.tile([128, 3 * NT], F32, name="gn", tag="gn")
                for ai in range(NT):
                    g_ps = ps_small.tile([128, 512], F32, name="g_ps", tag="smallps")
                    nc.tensor.transpose(
                        g_ps[:, 0:3], gt[0:3, ai * 128:(ai + 1) * 128],
                        I128[0:3, 0:3],
                    )
                    nc.vector.tensor_copy(gn[:, ai * 3:ai * 3 + 3], g_ps[:, 0:3])
                    hp_nat = ps_small.tile([128, 512], F32R, name="hp_nat",
                                           tag="smallps")
                    nc.tensor.transpose(
                        hp_nat[:, 0:D], hp[:, ai * 128:(ai + 1) * 128], I128_r
                    )
                    v = small.tile([128, D], F32, name="v", tag="v")
                    nc.gpsimd.tensor_scalar(
                        v[:], xn_sb[:, ai, :], gn[:, ai * 3 + 1:ai * 3 + 2],
                        None, OP.mult,
                    )
                    nc.vector.scalar_tensor_tensor(
                        out=w_all[:, ai * 128:(ai + 1) * 128],
                        in0=hp_nat[:, 0:D],
                        scalar=gn[:, ai * 3:ai * 3 + 1],
                        in1=v[:],
                        op0=OP.mult, op1=OP.add,
                    )
                nc.scalar.activation(t_all[:], w_all[:], AF.Tanh)
                for ai in range(NT):
                    nc.gpsimd.tensor_scalar(
                        out_sb[:, ai, :],
                        t_all[:, ai * 128:(ai + 1) * 128],
                        gn[:, ai * 3 + 2:ai * 3 + 3], None, OP.mult,
                    )
                nc.sync.dma_start(
                    out=out[g].rearrange("(t p) d -> p t d", p=128),
                    in_=out_sb[:],
                )
    nc.finalize()
    return nc


_CACHE = {}


def _get_program():
    if "nc" not in _CACHE:
        _CACHE["nc"] = _build_program()
    return _CACHE["nc"]


def _make_consts(inputs):
    A_ = np.asarray(inputs["A"], np.float32)
    return np.ascontiguousarray(np.concatenate([
        np.asarray(inputs["Ww"], np.float32).T,
        A_ + A_.T,
        np.eye(D, dtype=np.float32),
        np.asarray(inputs["Wb"], np.float32).reshape(D, 1),
        np.stack([inputs["wi_u"], inputs["wf_u"], inputs["wo_u"]],
                 axis=1).astype(np.float32),
        np.stack([inputs["wi_x"], inputs["wf_x"], inputs["wo_x"]],
                 axis=1).astype(np.float32),
    ], axis=1))


def kernel(x, adj, Ww, Wb, A, wi_u, wi_x, wf_u, wf_x, wo_u, wo_x):
    x = np.ascontiguousarray(np.asarray(x, dtype=np.float32))
    adj = np.asarray(adj, dtype=np.float32)

    bf16 = mybir.dt.np(BF16)
    # layout prep (host): transposes / stacking / dtype cast only
    xT_all = np.ascontiguousarray(x.transpose(0, 2, 1))           # [B, D, N]
    adjT_all = np.ascontiguousarray(
        adj.transpose(0, 2, 1)).astype(bf16)                      # [B, N, N]
    consts = _make_consts({
        "Ww": Ww, "A": A, "Wb": Wb,
        "wi_u": wi_u, "wf_u": wf_u, "wo_u": wo_u,
        "wi_x": wi_x, "wf_x": wf_x, "wo_x": wo_x,
    })

    nc = _get_program()
    in_maps = []
    for c in range(NCORES):
        s = slice(c * GPC, (c + 1) * GPC)
        in_maps.append({
            "xT": xT_all[s],
            "xn": x[s],
            "adjT": adjT_all[s],
            "consts": consts,
        })
    res = run_bass_kernel_spmd(nc, in_maps, list(range(NCORES)))
    out = np.empty((B, N, D), dtype=np.float32)
    for c in range(NCORES):
        out[c * GPC:(c + 1) * GPC] = res.results[c]["out"]
    return out
